# revision 27
# baseline (speedup 1.0000x reference)
"""GCNConv Trainium2 kernel: out = segment_sum(features[src], dst) @ W + b.

Strategy (8 NeuronCores, graph partitioned by destination node):
  - Host: partition the dst nodes across 8 cores (degree-balanced snake),
    49 slots of <=128 dst nodes per core.  The SWDGE byte rate (~134 GB/s
    per core) is the HW bottleneck, so the plan minimizes gathered bytes:
    one 256B bf16 descriptor per DISTINCT (slot, src) pair (duplicate edges
    fold into the one-hot as counts), descriptor streams packed with zero
    alignment padding (chunks crossing slot boundaries get one matmul per
    slot side).
  - Device (per core): dma_gather row descriptors (elem 128 bf16, lo/hi
    half tables for int16 indexing) in consumption order across 4 SWDGE
    queues; per gathered 128-desc chunk, a matmul with a host-precomputed
    one-hot block (streamed over the idle hardware DMA queues) accumulates
    agg.T per dst slot in PSUM; then out.T = W.T @ agg.T and a fused
    bias-add; DMA out.T tiles to DRAM.
  - Host: scatter per-core tile outputs back to [50000, 128].
"""

import os
import sys

for _p in ("/opt/trn_rl_repo",):
    if _p not in sys.path and os.path.isdir(_p):
        sys.path.insert(0, _p)

import numpy as np
import ml_dtypes

P = 128
N_NODES = 50000
N_EDGES = 640000
D = 128
NCORES = 8
HALF = 25000
NSLOT = (N_NODES // NCORES + P - 1) // P   # 49
GCHUNK = 8                    # chunks (of 128 descs) per dma_gather call;
                              # uniform so the lane-locked queue round-robin
                              # stays desc-balanced across the 4 SWDGE queues
NHEAD = 3                     # leading groups per stream with private index
                              # tiles (bridges the ~19us HWDGE bulk-load lag)
NQUEUES = 4
GBUFS = 8
SINGLE_PACKET = False

BF16 = ml_dtypes.bfloat16
STREAMS = ("L", "H")


# ---------------------------------------------------------------- host plan

def _assign_nodes(deg_lo, deg_hi):
    """Assign nodes to (core, slot, pos): degree-balanced snake over cores,
    then a greedy bin-pack of each core's nodes into slots balancing the
    per-slot (lo, hi) source-edge sums (the padded desc budgets are
    max-over-cores per slot, so flat slot sums minimize padding)."""
    deg = deg_lo + deg_hi
    order = np.argsort(-deg, kind="stable")
    snake = np.concatenate([np.arange(NCORES), np.arange(NCORES)[::-1]])
    core_of = np.empty(N_NODES, np.int32)
    core_of[order] = snake[np.arange(N_NODES) % (2 * NCORES)]
    slot_of = np.empty(N_NODES, np.int32)
    pos_of = np.empty(N_NODES, np.int32)
    node_lists = []
    for c in range(NCORES):
        nodes = order[core_of[order] == c]          # degree-sorted desc
        nn = len(nodes)
        lo, hi = deg_lo[nodes].astype(np.float64), deg_hi[nodes].astype(np.float64)
        TL = max(lo.sum() / NSLOT, 1e-9)
        TH = max(hi.sum() / NSLOT, 1e-9)
        sums = np.zeros((2, NSLOT))
        cnt = np.zeros(NSLOT, np.int64)
        sl = np.empty(nn, np.int64)
        cap_left = np.full(NSLOT, P, np.int64)
        # reserve capacity so every slot can reach ~nn/NSLOT nodes
        min_fill = nn // NSLOT
        for i in range(nn):
            score = np.maximum((sums[0] + lo[i]) / TL,
                               (sums[1] + hi[i]) / TH)
            score = score + 1e9 * (cap_left <= 0)
            s = int(np.argmin(score))
            sl[i] = s
            sums[0, s] += lo[i]
            sums[1, s] += hi[i]
            cap_left[s] -= 1
            cnt[s] += 1
        assert cnt.max() <= P
        pos = np.zeros(nn, np.int64)
        c2 = np.zeros(NSLOT, np.int64)
        for i in range(nn):
            pos[i] = c2[sl[i]]
            c2[sl[i]] += 1
        slot_of[nodes] = sl
        pos_of[nodes] = pos
        nl = [nodes[sl == s][np.argsort(pos[sl == s], kind="stable")]
              for s in range(NSLOT)]
        node_lists.append(nl)
    return core_of, slot_of, pos_of, node_lists


def plan(src, dst):
    src = np.asarray(src).astype(np.int64)
    dst = np.asarray(dst).astype(np.int64)
    deg_lo = np.bincount(dst[src < HALF], minlength=N_NODES)
    deg_hi = np.bincount(dst[src >= HALF], minlength=N_NODES)
    core_of, slot_of, pos_of, node_lists = _assign_nodes(deg_lo, deg_hi)

    # Per core / slot / stream: one desc per edge: (row_idx_int16, dst pos).
    # Desc order within a segment is shuffled: sorted/clustered source rows
    # put consecutive DMA reads in the same DRAM region, which serializes the
    # SDMA engines (measured 25% drain loss).
    descs = [{nm: [None] * NSLOT for nm in STREAMS} for _ in range(NCORES)]
    for c in range(NCORES):
        m = core_of[dst] == c
        s_c = src[m]
        slot_c = slot_of[dst[m]]
        dpos_c = pos_of[dst[m]]
        rng = np.random.default_rng(0xC0DE + c)
        perm = rng.permutation(len(s_c))
        s_s, sl_s, dp_s = s_c[perm], slot_c[perm], dpos_c[perm]
        lo = s_s < HALF
        for nm, msk in (("L", lo), ("H", ~lo)):
            rows = s_s[msk] - (0 if nm == "L" else HALF)
            sls = sl_s[msk]
            dps = dp_s[msk]
            for s in range(NSLOT):
                m2 = sls == s
                descs[c][nm][s] = list(zip(rows[m2].tolist(),
                                           dps[m2].tolist()))

    # shared per-slot stream lengths (max over cores)
    L = {nm: np.array([[len(descs[c][nm][s]) for s in range(NSLOT)]
                       for c in range(NCORES)]).max(axis=0)
         for nm in STREAMS}
    for s in range(NSLOT):
        if sum(int(L[nm][s]) for nm in STREAMS) == 0:
            L["L"][s] = 1
    start = {nm: np.concatenate([[0], np.cumsum(L[nm])]) for nm in STREAMS}
    K = {nm: -(-int(start[nm][-1]) // P) for nm in STREAMS}

    # shared matmul schedule: per slot, ordered list of (stream, chunk, ohcol)
    mms = []
    ohcol = 0
    for s in range(NSLOT):
        lst = []
        for nm in STREAMS:
            ln = int(L[nm][s])
            if ln == 0:
                continue
            st0 = int(start[nm][s])
            for j in range(st0 // P, (st0 + ln - 1) // P + 1):
                lst.append((nm, j, ohcol))
                ohcol += 1
        mms.append(lst)

    # dstl[c][p, mmcol] = dst position of desc p in that mm's chunk, or -1
    nmm = ohcol
    idxs = {nm: np.zeros((NCORES, K[nm] * P), np.int16) for nm in STREAMS}
    dstls = [np.full((P, nmm), -1.0, np.float32) for _ in range(NCORES)]
    for c in range(NCORES):
        dstl = dstls[c]
        for s in range(NSLOT):
            cmap = {(nm, j): col for (nm, j, col) in mms[s]}
            for nm in STREAMS:
                base = int(start[nm][s])
                for i, (ridx, dp) in enumerate(descs[c][nm][s]):
                    p = base + i
                    idxs[nm][c][p] = ridx
                    dstl[p % P, cmap[(nm, p // P)]] = dp
    return {
        "node_lists": node_lists,
        "L": L, "start": start, "K": K,
        "mms": mms, "NMM": nmm,
        "idxs": idxs, "dstls": dstls,
    }


def _groups(K):
    """Split K chunks into uniform GCHUNK gather groups."""
    return [(c, min(c + GCHUNK, K)) for c in range(0, K, GCHUNK)]


def _interleave(lens_by_stream, groups_by_stream):
    """Order gather groups by first consumption."""
    order = []
    nxt = {k: 0 for k in groups_by_stream}
    cur = {k: 0 for k in groups_by_stream}
    for s in range(NSLOT):
        for name in STREAMS:
            cur[name] += int(lens_by_stream[name][s])
            g = groups_by_stream[name]
            while nxt[name] < len(g) and g[nxt[name]][0] * P < cur[name]:
                order.append((name, g[nxt[name]]))
                nxt[name] += 1
    for name in STREAMS:
        g = groups_by_stream[name]
        while nxt[name] < len(g):
            order.append((name, g[nxt[name]]))
            nxt[name] += 1
    return order


def pack_gidx(idx, groups):
    """[K*128] desc-position-major int16 indices -> [128, K*8] dma_gather
    layout (index i of a group at [i%16, i//16], replicated over partitions
    16..127)."""
    K = len(idx) // P
    out = np.zeros((P, K * 8), np.int16)
    for c0, c1 in groups:
        g = idx[c0 * P:c1 * P]
        blk = g.reshape(-1, 16).T
        out[:, c0 * 8:c1 * 8] = np.tile(blk, (8, 1))
    return out


# ---------------------------------------------------------------- program

def build(pl, dbg=False):
    import concourse.bass as bass
    import concourse.mybir as mybir
    from concourse import bacc
    import concourse.tile as tile

    bf16, f32, i16 = mybir.dt.bfloat16, mybir.dt.float32, mybir.dt.int16
    K, NMM, mms = pl["K"], pl["NMM"], pl["mms"]

    nc = bacc.Bacc("TRN2", debug=dbg, num_swdge_queues=NQUEUES)
    flo = nc.dram_tensor("flo", [HALF, P], bf16, kind="ExternalInput")
    fhi = nc.dram_tensor("fhi", [N_NODES - HALF, P], bf16,
                         kind="ExternalInput")
    gxt = {nm: nc.dram_tensor("gidx" + nm, [P, K[nm] * 8], i16,
                              kind="ExternalInput") for nm in STREAMS}
    dstl = nc.dram_tensor("dstl", [P, NMM], bf16, kind="ExternalInput")
    iota = nc.dram_tensor("iota", [P, P], bf16, kind="ExternalInput")
    wmat = nc.dram_tensor("wmat", [P, P], bf16, kind="ExternalInput")
    bcol = nc.dram_tensor("bcol", [P, 1], f32, kind="ExternalInput")
    out = nc.dram_tensor("out", [P, NSLOT * P], f32, kind="ExternalOutput")

    groups = {nm: _groups(K[nm]) for nm in STREAMS}
    gorder = _interleave(pl["L"], groups)

    mm_rng = []  # contiguous mm-col range per slot
    for s in range(NSLOT):
        cols = [m[2] for m in mms[s]]
        mm_rng.append((min(cols), max(cols) + 1))

    with tile.TileContext(nc) as tc:
        with tc.tile_pool(name="const", bufs=1) as cp, \
             tc.tile_pool(name="gL", bufs=GBUFS) as pL, \
             tc.tile_pool(name="gH", bufs=GBUFS) as pH, \
             tc.tile_pool(name="ohp", bufs=6) as ohp, \
             tc.tile_pool(name="res", bufs=3) as resp, \
             tc.tile_pool(name="psA", bufs=4, space="PSUM") as psA, \
             tc.tile_pool(name="psB", bufs=2, space="PSUM") as psB:

            # Leading groups' indices go to private SBUF tiles via the Scalar
            # engine's (cold) HWDGE queue: Tile tracks dependencies per tile,
            # and HWDGE queues complete entries only near queue-empty, so
            # anything sharing a tile or queue with the bulk index load would
            # stall ~19us behind it.  NHEAD groups per stream (~15us of
            # drain) bridge that window.
            done = {nm: 0 for nm in STREAMS}
            head_t = {}
            for nm in STREAMS:
                for c0, c1 in groups[nm][:NHEAD]:
                    t = cp.tile([P, (c1 - c0) * 8], i16,
                                name=f"gidxh{nm}{c0}")
                    nc.scalar.dma_start(out=t[:],
                                        in_=gxt[nm][:, c0 * 8:c1 * 8])
                    head_t[(nm, c0)] = t
                    done[nm] = max(done[nm], c1)
            gidx_t = {nm: cp.tile([P, K[nm] * 8], i16, name="gidxt" + nm)
                      for nm in STREAMS}

            tabs = {"L": flo, "H": fhi}
            pools = {"L": pL, "H": pH}
            st = {nm: {"tiles": {}, "g": 0} for nm in STREAMS}
            qcount = [0]

            def fetch(nm):
                S = st[nm]
                gi = S["g"]
                c0, c1 = groups[nm][gi]
                n = c1 - c0
                t = pools[nm].tile([P, n * P], mybir.dt.bfloat16,
                                   tag="g" + nm)
                ht = head_t.get((nm, c0))
                idxs_ap = ht[:] if ht is not None \
                    else gidx_t[nm][:, c0 * 8:c1 * 8]
                nc.gpsimd.dma_gather(
                    out_ap=t[:].rearrange("p (g d) -> p g d", d=P),
                    in_ap=tabs[nm][:],
                    idxs_ap=idxs_ap,
                    num_idxs=n * P,
                    num_idxs_reg=n * P,
                    elem_size=P,
                    single_packet=SINGLE_PACKET,
                    queue_num=qcount[0] % NQUEUES,
                )
                qcount[0] += 1
                S["tiles"][gi] = (t, c0, c1)
                S["g"] += 1

            nfirst = 0
            for nm, (c0, c1) in gorder:
                if (nm, c0) not in head_t:
                    break
                fetch(nm)
                nfirst += 1

            # dstl + iota early (one-hot builds depend on them) and
            # weights/bias on the scalar queue; the bulk index load gets the
            # sync queue to itself so it completes quickly
            dstl_t = cp.tile([P, NMM], bf16)
            nc.scalar.dma_start(out=dstl_t[:], in_=dstl[:])
            iota_t = cp.tile([P, P], bf16)
            nc.scalar.dma_start(out=iota_t[:], in_=iota[:])
            w_t = cp.tile([P, P], bf16)
            nc.scalar.dma_start(out=w_t[:], in_=wmat[:])
            b_t = cp.tile([P, 1], f32)
            nc.scalar.dma_start(out=b_t[:], in_=bcol[:])
            for nm in STREAMS:
                c0 = done[nm]
                if c0 < K[nm]:
                    nc.sync.dma_start(out=gidx_t[nm][:, c0 * 8:K[nm] * 8],
                                      in_=gxt[nm][:, c0 * 8:K[nm] * 8])

            for nm, _ in gorder[nfirst:]:
                fetch(nm)

            def find_tile(nm, j):
                for gi, (t, c0, c1) in st[nm]["tiles"].items():
                    if c0 <= j < c1:
                        return t, c0
                raise KeyError((nm, j))

            def onehot(s):
                """one tensor_tensor -> [P, k*128] bf16 one-hot blocks for
                the k matmuls of slot s."""
                m0, m1 = mm_rng[s]
                k = m1 - m0
                oh = ohp.tile([P, k * P], mybir.dt.bfloat16, tag="oh")
                in0 = iota_t[:].rearrange("p (k f) -> p k f", k=1) \
                    .broadcast_to([P, k, P])
                in1 = dstl_t[:, m0:m1].rearrange("p (k o) -> p k o", o=1) \
                    .broadcast_to([P, k, P])
                outv = oh[:].rearrange("p (k f) -> p k f", k=k)
                import concourse.mybir as mybir_
                nc.vector.tensor_tensor(out=outv, in0=in0, in1=in1,
                                        op=mybir_.AluOpType.is_equal)
                return oh, m0

            for s in range(NSLOT):
                ps_agg = psA.tile([P, P], f32, tag="agg")
                oh_t, m0 = onehot(s)
                n = len(mms[s])
                for k, (nm, j, col) in enumerate(mms[s]):
                    t, c0 = find_tile(nm, j)
                    lo = (j - c0) * P
                    oc = (col - m0) * P
                    nc.tensor.matmul(
                        out=ps_agg[:],
                        lhsT=t[:, lo:lo + P],
                        rhs=oh_t[:, oc:oc + P],
                        start=(k == 0), stop=(k == n - 1),
                    )
                aggT = resp.tile([P, P], mybir.dt.bfloat16, tag="aggT")
                nc.scalar.copy(out=aggT[:], in_=ps_agg[:])
                ps_out = psB.tile([P, P], f32, tag="out")
                nc.tensor.matmul(out=ps_out[:], lhsT=w_t[:], rhs=aggT[:],
                                 start=True, stop=True)
                o_sb = resp.tile([P, P], f32, tag="osb")
                nc.scalar.activation(
                    out=o_sb[:], in_=ps_out[:],
                    func=mybir.ActivationFunctionType.Identity,
                    bias=b_t[:, 0:1],
                )
                nc.sync.dma_start(out=out[:, s * P:(s + 1) * P], in_=o_sb[:])

    # Spread gathers across SWDGE queues (queue must be a function of the
    # scheduled DMASW lane).
    for inst in nc.inst_map.values():
        if isinstance(inst, mybir.InstDMAGatherAnt):
            proc = inst.bass_scheduled_proc
            if proc is not None and 11 <= proc <= 18:
                inst.queue_num = (proc - 11) % NQUEUES

    nc.compile()
    return nc


# ---------------------------------------------------------------- in_maps

def make_in_maps(features, W, b, pl):
    f16 = np.ascontiguousarray(features).astype(BF16)
    w_np = np.asarray(W, np.float32).astype(BF16)
    b_np = np.asarray(b, np.float32).reshape(1, D).T.copy()
    iota_np = np.tile(np.arange(P, dtype=np.float32)[None, :],
                      (P, 1)).astype(BF16)
    groups = {nm: _groups(pl["K"][nm]) for nm in STREAMS}
    in_maps = []
    for c in range(NCORES):
        m = {
            "flo": f16[:HALF],
            "fhi": f16[HALF:],
            "dstl": np.ascontiguousarray(pl["dstls"][c]).astype(BF16),
            "iota": iota_np,
            "wmat": w_np,
            "bcol": b_np,
        }
        for nm in STREAMS:
            m["gidx" + nm] = pack_gidx(pl["idxs"][nm][c], groups[nm])
        in_maps.append(m)
    return in_maps


def unshard(outs, node_lists):
    full = np.zeros((N_NODES, D), np.float32)
    for c in range(NCORES):
        oT = np.asarray(outs[c]["out"], np.float32)
        for s in range(NSLOT):
            ns = node_lists[c][s]
            if len(ns) == 0:
                continue
            full[ns, :] = oT[:, s * P:s * P + len(ns)].T
    return full


# ---------------------------------------------------------------- entry

_CACHE = {}


def kernel(features, src, dst, W, b):
    from concourse.bass_utils import run_bass_kernel_spmd

    pl = plan(src, dst)
    key = tuple(tuple(pl["L"][nm]) for nm in STREAMS)
    if key not in _CACHE:
        _CACHE[key] = build(pl)
    nc = _CACHE[key]
    in_maps = make_in_maps(features, W, b, pl)
    last = None
    for _ in range(3):  # retry: a previously wedged pool device can fail a load
        try:
            res = run_bass_kernel_spmd(nc, in_maps, core_ids=list(range(NCORES)))
            return unshard(res.results, pl["node_lists"])
        except Exception as e:  # noqa: BLE001
            last = e
    raise last


# revision 31
# speedup vs baseline: 1.0485x; 1.0485x over previous
"""GCNConv Trainium2 kernel: out = segment_sum(features[src], dst) @ W + b.

Strategy (8 NeuronCores, graph partitioned by destination node):
  - Host: partition the dst nodes across 8 cores (degree-balanced snake),
    49 slots of <=128 dst nodes per core.  The SWDGE byte rate (~134 GB/s
    per core) is the HW bottleneck, so the plan minimizes gathered bytes:
    one 256B bf16 descriptor per DISTINCT (slot, src) pair (duplicate edges
    fold into the one-hot as counts), descriptor streams packed with zero
    alignment padding (chunks crossing slot boundaries get one matmul per
    slot side).
  - Device (per core): dma_gather row descriptors (elem 128 bf16, lo/hi
    half tables for int16 indexing) in consumption order across 4 SWDGE
    queues; per gathered 128-desc chunk, a matmul with a host-precomputed
    one-hot block (streamed over the idle hardware DMA queues) accumulates
    agg.T per dst slot in PSUM; then out.T = W.T @ agg.T and a fused
    bias-add; DMA out.T tiles to DRAM.
  - Host: scatter per-core tile outputs back to [50000, 128].
"""

import os
import sys

for _p in ("/opt/trn_rl_repo",):
    if _p not in sys.path and os.path.isdir(_p):
        sys.path.insert(0, _p)

import numpy as np
import ml_dtypes

P = 128
N_NODES = 50000
N_EDGES = 640000
D = 128
NCORES = 8
HALF = 25000
NSLOT = (N_NODES // NCORES + P - 1) // P   # 49
GCHUNK = 16                   # chunks (of 128 descs) per dma_gather call;
                              # uniform so the lane-locked queue round-robin
                              # stays desc-balanced across the 4 SWDGE queues
NHEAD = 2                     # leading groups per stream with a private index
                              # tile (bridges the ~19us HWDGE bulk-load lag)
NQUEUES = 4
GBUFS = 8
SINGLE_PACKET = False

BF16 = ml_dtypes.bfloat16
STREAMS = ("L", "H")


# ---------------------------------------------------------------- host plan

def _assign_nodes(deg_lo, deg_hi):
    """Assign nodes to (core, slot, pos): degree-balanced snake over cores,
    then a greedy bin-pack of each core's nodes into slots balancing the
    per-slot (lo, hi) source-edge sums (the padded desc budgets are
    max-over-cores per slot, so flat slot sums minimize padding)."""
    deg = deg_lo + deg_hi
    order = np.argsort(-deg, kind="stable")
    snake = np.concatenate([np.arange(NCORES), np.arange(NCORES)[::-1]])
    core_of = np.empty(N_NODES, np.int32)
    core_of[order] = snake[np.arange(N_NODES) % (2 * NCORES)]
    slot_of = np.empty(N_NODES, np.int32)
    pos_of = np.empty(N_NODES, np.int32)
    node_lists = []
    for c in range(NCORES):
        nodes = order[core_of[order] == c]          # degree-sorted desc
        nn = len(nodes)
        lo, hi = deg_lo[nodes].astype(np.float64), deg_hi[nodes].astype(np.float64)
        TL = max(lo.sum() / NSLOT, 1e-9)
        TH = max(hi.sum() / NSLOT, 1e-9)
        sums = np.zeros((2, NSLOT))
        cnt = np.zeros(NSLOT, np.int64)
        sl = np.empty(nn, np.int64)
        cap_left = np.full(NSLOT, P, np.int64)
        # reserve capacity so every slot can reach ~nn/NSLOT nodes
        min_fill = nn // NSLOT
        for i in range(nn):
            score = np.maximum((sums[0] + lo[i]) / TL,
                               (sums[1] + hi[i]) / TH)
            score = score + 1e9 * (cap_left <= 0)
            s = int(np.argmin(score))
            sl[i] = s
            sums[0, s] += lo[i]
            sums[1, s] += hi[i]
            cap_left[s] -= 1
            cnt[s] += 1
        assert cnt.max() <= P
        pos = np.zeros(nn, np.int64)
        c2 = np.zeros(NSLOT, np.int64)
        for i in range(nn):
            pos[i] = c2[sl[i]]
            c2[sl[i]] += 1
        slot_of[nodes] = sl
        pos_of[nodes] = pos
        nl = [nodes[sl == s][np.argsort(pos[sl == s], kind="stable")]
              for s in range(NSLOT)]
        node_lists.append(nl)
    return core_of, slot_of, pos_of, node_lists


def plan(src, dst):
    src = np.asarray(src).astype(np.int64)
    dst = np.asarray(dst).astype(np.int64)
    deg_lo = np.bincount(dst[src < HALF], minlength=N_NODES)
    deg_hi = np.bincount(dst[src >= HALF], minlength=N_NODES)
    core_of, slot_of, pos_of, node_lists = _assign_nodes(deg_lo, deg_hi)

    # Per core / slot / stream: one desc per edge: (row_idx_int16, dst pos).
    # Desc order within a segment is shuffled: sorted/clustered source rows
    # put consecutive DMA reads in the same DRAM region, which serializes the
    # SDMA engines (measured 25% drain loss).
    descs = [{nm: [None] * NSLOT for nm in STREAMS} for _ in range(NCORES)]
    for c in range(NCORES):
        m = core_of[dst] == c
        s_c = src[m]
        slot_c = slot_of[dst[m]]
        dpos_c = pos_of[dst[m]]
        rng = np.random.default_rng(0xC0DE + c)
        perm = rng.permutation(len(s_c))
        s_s, sl_s, dp_s = s_c[perm], slot_c[perm], dpos_c[perm]
        lo = s_s < HALF
        for nm, msk in (("L", lo), ("H", ~lo)):
            rows = s_s[msk] - (0 if nm == "L" else HALF)
            sls = sl_s[msk]
            dps = dp_s[msk]
            for s in range(NSLOT):
                m2 = sls == s
                descs[c][nm][s] = list(zip(rows[m2].tolist(),
                                           dps[m2].tolist()))

    # shared per-slot stream lengths (max over cores)
    L = {nm: np.array([[len(descs[c][nm][s]) for s in range(NSLOT)]
                       for c in range(NCORES)]).max(axis=0)
         for nm in STREAMS}
    for s in range(NSLOT):
        if sum(int(L[nm][s]) for nm in STREAMS) == 0:
            L["L"][s] = 1
    start = {nm: np.concatenate([[0], np.cumsum(L[nm])]) for nm in STREAMS}
    K = {nm: -(-int(start[nm][-1]) // P) for nm in STREAMS}

    # shared matmul schedule: per slot, ordered list of (stream, chunk, ohcol)
    mms = []
    ohcol = 0
    for s in range(NSLOT):
        lst = []
        for nm in STREAMS:
            ln = int(L[nm][s])
            if ln == 0:
                continue
            st0 = int(start[nm][s])
            for j in range(st0 // P, (st0 + ln - 1) // P + 1):
                lst.append((nm, j, ohcol))
                ohcol += 1
        mms.append(lst)

    # dstl[c][p, mmcol] = dst position of desc p in that mm's chunk, or -1
    nmm = ohcol
    idxs = {nm: np.zeros((NCORES, K[nm] * P), np.int16) for nm in STREAMS}
    dstls = [np.full((P, nmm), -1.0, np.float32) for _ in range(NCORES)]
    for c in range(NCORES):
        dstl = dstls[c]
        for s in range(NSLOT):
            cmap = {(nm, j): col for (nm, j, col) in mms[s]}
            for nm in STREAMS:
                base = int(start[nm][s])
                for i, (ridx, dp) in enumerate(descs[c][nm][s]):
                    p = base + i
                    idxs[nm][c][p] = ridx
                    dstl[p % P, cmap[(nm, p // P)]] = dp
    return {
        "node_lists": node_lists,
        "L": L, "start": start, "K": K,
        "mms": mms, "NMM": nmm,
        "idxs": idxs, "dstls": dstls,
    }


def _groups(K):
    """Split K chunks into uniform GCHUNK gather groups."""
    return [(c, min(c + GCHUNK, K)) for c in range(0, K, GCHUNK)]


def _interleave(lens_by_stream, groups_by_stream):
    """Order gather groups by first consumption."""
    order = []
    nxt = {k: 0 for k in groups_by_stream}
    cur = {k: 0 for k in groups_by_stream}
    for s in range(NSLOT):
        for name in STREAMS:
            cur[name] += int(lens_by_stream[name][s])
            g = groups_by_stream[name]
            while nxt[name] < len(g) and g[nxt[name]][0] * P < cur[name]:
                order.append((name, g[nxt[name]]))
                nxt[name] += 1
    for name in STREAMS:
        g = groups_by_stream[name]
        while nxt[name] < len(g):
            order.append((name, g[nxt[name]]))
            nxt[name] += 1
    return order


def pack_gidx(idx, groups):
    """[K*128] desc-position-major int16 indices -> [128, K*8] dma_gather
    layout (index i of a group at [i%16, i//16], replicated over partitions
    16..127)."""
    K = len(idx) // P
    out = np.zeros((P, K * 8), np.int16)
    for c0, c1 in groups:
        g = idx[c0 * P:c1 * P]
        blk = g.reshape(-1, 16).T
        out[:, c0 * 8:c1 * 8] = np.tile(blk, (8, 1))
    return out


# ---------------------------------------------------------------- program

def build(pl, dbg=False):
    import concourse.bass as bass
    import concourse.mybir as mybir
    from concourse import bacc
    import concourse.tile as tile

    bf16, f32, i16 = mybir.dt.bfloat16, mybir.dt.float32, mybir.dt.int16
    K, NMM, mms = pl["K"], pl["NMM"], pl["mms"]

    nc = bacc.Bacc("TRN2", debug=dbg, num_swdge_queues=NQUEUES)
    flo = nc.dram_tensor("flo", [HALF, P], bf16, kind="ExternalInput")
    fhi = nc.dram_tensor("fhi", [N_NODES - HALF, P], bf16,
                         kind="ExternalInput")
    gxt = {nm: nc.dram_tensor("gidx" + nm, [P, K[nm] * 8], i16,
                              kind="ExternalInput") for nm in STREAMS}
    dstl = nc.dram_tensor("dstl", [P, NMM], bf16, kind="ExternalInput")
    iota = nc.dram_tensor("iota", [P, P], bf16, kind="ExternalInput")
    wmat = nc.dram_tensor("wmat", [P, P], bf16, kind="ExternalInput")
    bcol = nc.dram_tensor("bcol", [P, 1], f32, kind="ExternalInput")
    out = nc.dram_tensor("out", [P, NSLOT * P], f32, kind="ExternalOutput")

    groups = {nm: _groups(K[nm]) for nm in STREAMS}
    gorder = _interleave(pl["L"], groups)

    mm_rng = []  # contiguous mm-col range per slot
    for s in range(NSLOT):
        cols = [m[2] for m in mms[s]]
        mm_rng.append((min(cols), max(cols) + 1))

    with tile.TileContext(nc) as tc:
        with tc.tile_pool(name="const", bufs=1) as cp, \
             tc.tile_pool(name="gL", bufs=GBUFS) as pL, \
             tc.tile_pool(name="gH", bufs=GBUFS) as pH, \
             tc.tile_pool(name="ohp", bufs=6) as ohp, \
             tc.tile_pool(name="res", bufs=3) as resp, \
             tc.tile_pool(name="psA", bufs=4, space="PSUM") as psA, \
             tc.tile_pool(name="psB", bufs=2, space="PSUM") as psB:

            # Leading groups' indices go to a private SBUF tile per stream
            # (one wide DMA each, big per-partition rows) on the Sync HWDGE
            # queue, which starts earliest; the bulk index load moves to the
            # Scalar queue.  Tile tracks dependencies per tile and HWDGE
            # queues complete entries only near queue-empty, so anything
            # sharing a tile or queue with the bulk load stalls ~19us.
            done = {nm: 0 for nm in STREAMS}
            head_t = {}
            hc = {nm: min(NHEAD * GCHUNK, K[nm]) for nm in STREAMS}
            for nm in STREAMS:
                t = cp.tile([P, hc[nm] * 8], i16, name="gidxhd" + nm)
                nc.sync.dma_start(out=t[:], in_=gxt[nm][:, :hc[nm] * 8])
                for c0, c1 in groups[nm][:NHEAD]:
                    head_t[(nm, c0)] = (t, c0)
                    done[nm] = max(done[nm], c1)
            gidx_t = {nm: cp.tile([P, K[nm] * 8], i16, name="gidxt" + nm)
                      for nm in STREAMS}

            tabs = {"L": flo, "H": fhi}
            pools = {"L": pL, "H": pH}
            st = {nm: {"tiles": {}, "g": 0} for nm in STREAMS}
            qcount = [0]

            def fetch(nm):
                S = st[nm]
                gi = S["g"]
                c0, c1 = groups[nm][gi]
                n = c1 - c0
                t = pools[nm].tile([P, n * P], mybir.dt.bfloat16,
                                   tag="g" + nm)
                ht = head_t.get((nm, c0))
                idxs_ap = ht[0][:, c0 * 8:c1 * 8] if ht is not None \
                    else gidx_t[nm][:, c0 * 8:c1 * 8]
                nc.gpsimd.dma_gather(
                    out_ap=t[:].rearrange("p (g d) -> p g d", d=P),
                    in_ap=tabs[nm][:],
                    idxs_ap=idxs_ap,
                    num_idxs=n * P,
                    num_idxs_reg=n * P,
                    elem_size=P,
                    single_packet=SINGLE_PACKET,
                    queue_num=qcount[0] % NQUEUES,
                )
                qcount[0] += 1
                S["tiles"][gi] = (t, c0, c1)
                S["g"] += 1

            nfirst = 0
            for nm, (c0, c1) in gorder:
                if (nm, c0) not in head_t:
                    break
                fetch(nm)
                nfirst += 1

            # dstl + iota early on the sync queue (one-hot builds depend on
            # them); weights/bias too; the bulk index load goes to the
            # scalar queue so the sync queue empties (and completes) fast
            dstl_t = cp.tile([P, NMM], bf16)
            nc.sync.dma_start(out=dstl_t[:], in_=dstl[:])
            iota_t = cp.tile([P, P], bf16)
            nc.sync.dma_start(out=iota_t[:], in_=iota[:])
            w_t = cp.tile([P, P], bf16)
            nc.sync.dma_start(out=w_t[:], in_=wmat[:])
            b_t = cp.tile([P, 1], f32)
            nc.sync.dma_start(out=b_t[:], in_=bcol[:])
            for nm in STREAMS:
                c0 = done[nm]
                if c0 < K[nm]:
                    nc.scalar.dma_start(out=gidx_t[nm][:, c0 * 8:K[nm] * 8],
                                        in_=gxt[nm][:, c0 * 8:K[nm] * 8])

            for nm, _ in gorder[nfirst:]:
                fetch(nm)

            def find_tile(nm, j):
                for gi, (t, c0, c1) in st[nm]["tiles"].items():
                    if c0 <= j < c1:
                        return t, c0
                raise KeyError((nm, j))

            def onehot(s):
                """one tensor_tensor -> [P, k*128] bf16 one-hot blocks for
                the k matmuls of slot s."""
                m0, m1 = mm_rng[s]
                k = m1 - m0
                oh = ohp.tile([P, k * P], mybir.dt.bfloat16, tag="oh")
                in0 = iota_t[:].rearrange("p (k f) -> p k f", k=1) \
                    .broadcast_to([P, k, P])
                in1 = dstl_t[:, m0:m1].rearrange("p (k o) -> p k o", o=1) \
                    .broadcast_to([P, k, P])
                outv = oh[:].rearrange("p (k f) -> p k f", k=k)
                import concourse.mybir as mybir_
                nc.vector.tensor_tensor(out=outv, in0=in0, in1=in1,
                                        op=mybir_.AluOpType.is_equal)
                return oh, m0

            for s in range(NSLOT):
                ps_agg = psA.tile([P, P], f32, tag="agg")
                oh_t, m0 = onehot(s)
                n = len(mms[s])
                for k, (nm, j, col) in enumerate(mms[s]):
                    t, c0 = find_tile(nm, j)
                    lo = (j - c0) * P
                    oc = (col - m0) * P
                    nc.tensor.matmul(
                        out=ps_agg[:],
                        lhsT=t[:, lo:lo + P],
                        rhs=oh_t[:, oc:oc + P],
                        start=(k == 0), stop=(k == n - 1),
                    )
                aggT = resp.tile([P, P], mybir.dt.bfloat16, tag="aggT")
                nc.scalar.copy(out=aggT[:], in_=ps_agg[:])
                ps_out = psB.tile([P, P], f32, tag="out")
                nc.tensor.matmul(out=ps_out[:], lhsT=w_t[:], rhs=aggT[:],
                                 start=True, stop=True)
                o_sb = resp.tile([P, P], f32, tag="osb")
                nc.scalar.activation(
                    out=o_sb[:], in_=ps_out[:],
                    func=mybir.ActivationFunctionType.Identity,
                    bias=b_t[:, 0:1],
                )
                nc.sync.dma_start(out=out[:, s * P:(s + 1) * P], in_=o_sb[:])

    # Spread gathers across SWDGE queues (queue must be a function of the
    # scheduled DMASW lane).
    for inst in nc.inst_map.values():
        if isinstance(inst, mybir.InstDMAGatherAnt):
            proc = inst.bass_scheduled_proc
            if proc is not None and 11 <= proc <= 18:
                inst.queue_num = (proc - 11) % NQUEUES

    nc.compile()
    return nc


# ---------------------------------------------------------------- in_maps

def make_in_maps(features, W, b, pl):
    f16 = np.ascontiguousarray(features).astype(BF16)
    w_np = np.asarray(W, np.float32).astype(BF16)
    b_np = np.asarray(b, np.float32).reshape(1, D).T.copy()
    iota_np = np.tile(np.arange(P, dtype=np.float32)[None, :],
                      (P, 1)).astype(BF16)
    groups = {nm: _groups(pl["K"][nm]) for nm in STREAMS}
    in_maps = []
    for c in range(NCORES):
        m = {
            "flo": f16[:HALF],
            "fhi": f16[HALF:],
            "dstl": np.ascontiguousarray(pl["dstls"][c]).astype(BF16),
            "iota": iota_np,
            "wmat": w_np,
            "bcol": b_np,
        }
        for nm in STREAMS:
            m["gidx" + nm] = pack_gidx(pl["idxs"][nm][c], groups[nm])
        in_maps.append(m)
    return in_maps


def unshard(outs, node_lists):
    full = np.zeros((N_NODES, D), np.float32)
    for c in range(NCORES):
        oT = np.asarray(outs[c]["out"], np.float32)
        for s in range(NSLOT):
            ns = node_lists[c][s]
            if len(ns) == 0:
                continue
            full[ns, :] = oT[:, s * P:s * P + len(ns)].T
    return full


# ---------------------------------------------------------------- entry

_CACHE = {}


def kernel(features, src, dst, W, b):
    from concourse.bass_utils import run_bass_kernel_spmd

    pl = plan(src, dst)
    key = tuple(tuple(pl["L"][nm]) for nm in STREAMS)
    if key not in _CACHE:
        _CACHE[key] = build(pl)
    nc = _CACHE[key]
    in_maps = make_in_maps(features, W, b, pl)
    last = None
    for _ in range(3):  # retry: a previously wedged pool device can fail a load
        try:
            res = run_bass_kernel_spmd(nc, in_maps, core_ids=list(range(NCORES)))
            return unshard(res.results, pl["node_lists"])
        except Exception as e:  # noqa: BLE001
            last = e
    raise last
